# revision 53
# baseline (speedup 1.0000x reference)
"""Trainium2 Bass kernel for CSSM (Mamba-style 2D selective scan block).

Sharding: 8 cores = 4 batch x 2 d_inner-halves. Each core computes the
full front-end for its batch element, the selective scan for its 96
d_inner channels x 16 states, and a partial output projection. The host
sums the two partial outputs per batch element.

Key structure:
- The 1x1 input projection is folded into the 3x3 depthwise-ish conv on
  the host (W_eff[o,c,tap] = sum_i dconv_w[o,i,tap] proj_w[i,c]), so the
  conv needs 9 matmuls per PSUM unit instead of 18 and the proj
  stage disappears (PE matmul cost is K-independent).
- The causal depthwise conv1d is folded into in_proj the same way
  (4-tap full conv applied straight to the conv trunk).
- dA_n = exp(-(n+1) delta): the first NACT states come from the scalar
  engine (exp, bf16 out); the rest are bf16 products dA_{NACT-1} *
  dA_{n-NACT} on the vector engine (2x mode).
- Scans (tensor_tensor_scan, always 1x, DVE-only in this backend) and
  tmp products run on DVE where bf16 muls run 2x; most dBu products are
  issued up-front on GPSIMD(Pool).
- D*u enters the y accumulation as a diag(D) matmul into the same PSUM
  banks as the per-state ident matmuls; the gate multiply reads PSUM
  on DVE.
- Scalar-engine ops are grouped so the activation table switches only
  twice per block (Silu group vs Exp/Ln group).
"""
import sys

sys.path.insert(0, "/opt/trn_rl_repo")

import numpy as np

C = 96            # d_model; also channels per d_inner half
DI = 192          # d_inner
NST = 16          # d_state
DTR = 6           # dt_rank
HH = 64
WW = 64
L = HH * WW       # 4096
T = 512           # matmul moving-dim chunk
T2 = 1024         # block chunk
PW = WW + 2       # 66: padded row width for the 3x3 conv
G = 68            # left guard of the padded conv buffer
PADLEN = G + (HH + 2) * PW + 68
BLOCKS = ((0, 256), (256, 768), (1024, 1024), (2048, 1024), (3072, 1024))
NACT = 16         # states 0..NACT-1: dA by scalar-engine exp (rest: DVE muls)
POOL_DBU = frozenset(range(5, 16))   # states whose dBu mul runs on gpsimd

_CACHE = {}


def _emit(tc, nc, mybir, dram):
    from contextlib import ExitStack

    from concourse import bass

    f32 = mybir.dt.float32
    bf16 = mybir.dt.bfloat16
    AF = mybir.ActivationFunctionType
    OP = mybir.AluOpType

    def mmacc(out, pairs, start=True, stop=True, ncols=None):
        """Matmul with free-dim split at 512-col PSUM-bank boundaries."""
        n = ncols if ncols is not None else out.shape[-1]
        bounds = list(range(0, n, T)) + [n]
        for c0, c1 in zip(bounds[:-1], bounds[1:]):
            for i, (lh, rh) in enumerate(pairs):
                nc.tensor.matmul(out[:, c0:c1], lh, rh[:, c0:c1],
                                 start=start and i == 0,
                                 stop=stop and i == len(pairs) - 1)

    with ExitStack() as ctx:
        ec = ctx.enter_context
        consts = ec(tc.tile_pool(name="consts", bufs=1))
        persist = ec(tc.tile_pool(name="persist", bufs=1))
        dpool = ec(tc.tile_pool(name="dpool", bufs=1, space="DRAM"))
        fw = ec(tc.tile_pool(name="fw", bufs=1))
        pxc2 = ec(tc.tile_pool(name="pxc2", bufs=2))
        pxca = ec(tc.tile_pool(name="pxca", bufs=3))
        pxcb = ec(tc.tile_pool(name="pxcb", bufs=2))
        psz = ec(tc.tile_pool(name="psz", bufs=2))
        pxd = ec(tc.tile_pool(name="pxd", bufs=2))
        ped = ec(tc.tile_pool(name="ped", bufs=2))
        dl = ec(tc.tile_pool(name="dl", bufs=2))
        dap = ec(tc.tile_pool(name="dap", bufs=9))
        lp = ec(tc.tile_pool(name="lp", bufs=3))
        gbp = ec(tc.tile_pool(name="gbp", bufs=len(POOL_DBU) + 2))
        hp = ec(tc.tile_pool(name="hp", bufs=5))
        bc = ec(tc.tile_pool(name="bc", bufs=18))
        tl = ec(tc.tile_pool(name="tl", bufs=2))
        pbig = ec(tc.tile_pool(name="pbig", bufs=2, space="PSUM"))
        pbank = ec(tc.tile_pool(name="pbank", bufs=2, space="PSUM"))
        psy = ec(tc.tile_pool(name="psy", bufs=2, space="PSUM"))

        def cload(name, shape, dtype=f32, rearr=None, pool=None):
            t = (pool or consts).tile(list(shape), dtype, tag=name)
            src = dram[name]
            if rearr is not None:
                src = src.rearrange(rearr)
            nc.sync.dma_start(t[:], src)
            return t

        xp1 = persist.tile([C, PADLEN], bf16, tag="xp1")
        # Zero only the guard/pad columns (disjoint from the x rows the DMAs
        # write) so the block-0 x DMA is not ordered after a full-buffer
        # memset. Three small memsets: head guard + top pad row, the two pad
        # columns between consecutive rows, bottom pad row + tail guard.
        nc.gpsimd.memset(xp1[:, 0:G + PW + 1], 0.0)
        mid = xp1[:, G + PW + 65: G + 65 * PW + 65]
        nc.gpsimd.memset(mid.rearrange("p (r w) -> p r w", w=PW)[:, :, 0:2], 0.0)
        nc.gpsimd.memset(xp1[:, G + 65 * PW + 1: PADLEN], 0.0)

        # fill-critical loads first: dconv weights, then the block-0 x rows
        wde_sb = cload("wde", (C, 9, C), bf16, "t k m -> k t m", pool=fw)
        cs0, bw0 = BLOCKS[0]
        nr0 = bw0 // WW
        dst0 = xp1[:, G + PW + 1: G + (nr0 + 1) * PW + 1]
        nc.sync.dma_start(dst0.rearrange("p (r w) -> p r w", w=PW)[:, :, 0:WW],
                          dram["x"][:, 0:bw0].rearrange("p (r w) -> p r w",
                                                        w=WW))
        w1e_sb = cload("w1e", (C, 8, C), bf16, "g t k m -> k (g t) m", pool=fw)
        winz_sb = cload("winz", (C, C), bf16, pool=fw)
        wxp_sb = cload("wxp", (C, 2, 38), bf16, "g k m -> k g m", pool=fw)
        wdt_sb = cload("wdt", (32, C), bf16)
        bdt_sb = cload("bdt", (C, 1))
        wout_sb = cload("wout", (C, C), bf16)
        b1d_sb = cload("b1d", (C, 2))
        aneg_sb = cload("aneg", (C, NST))       # -exp(A_log), local rows
        ident_sb = cload("ident", (C, C), bf16)
        ddiag_sb = cload("ddiag", (C, C), bf16)

        carry = persist.tile([C, NST], f32, tag="carry")
        xdd = dpool.tile([38, L], bf16, tag="xdd")

        state = {"xc2_prev": None}

        def _front_pieces(s):
            """Front-end for block s as 16 pieces, interleaved with the scan
            of block s-1 so each in-order engine alternates between the two
            stages at fine grain."""
            cs, bw = BLOCKS[s]
            ce = cs + bw
            chs = [(c0, min(T, bw - c0)) for c0 in range(0, bw, T)]
            nrow = bw // WW
            units = []
            ro = 0
            while ro < nrow:
                units.append((ro, min(7, nrow - ro)))
                ro += 7
            xc2 = pxc2.tile([C, 3 + T2], bf16, tag="xc2", name=f"xc2_{s}")
            holder = {}

            def p_dconv(u):
                def go():
                    if u >= len(units):
                        return
                    if u == 0:
                        if s == 0:
                            nc.vector.memset(xc2[:, 0:3], 0.0)
                        else:
                            pw = BLOCKS[s - 1][1]
                            nc.vector.tensor_copy(
                                xc2[:, 0:3], state["xc2_prev"][:, pw:pw + 3])
                        state["xc2_prev"] = xc2
                    ro, rows = units[u]
                    r0 = cs // WW + ro
                    cols = rows * PW
                    base = G + (r0 + 1) * PW
                    psd = pbank.tile([C, T], f32, tag="pbank",
                                     name=f"dconv_{s}_{u}")
                    pairs = []
                    for tap in range(9):
                        dy, dx = tap // 3, tap % 3
                        shift = (dy - 1) * PW + (dx - 1)
                        pairs.append((wde_sb[:, tap, :],
                                      xp1[:, base + shift: base + shift + cols]))
                    mmacc(psd[:, :cols], pairs, ncols=cols)
                    srcv = psd[:, :cols].rearrange("p (r w) -> p r w",
                                                   w=PW)[:, :, 1:65]
                    dstv = xc2[:, 3 + ro * WW: 3 + (ro + rows) * WW]
                    nc.scalar.activation(
                        dstv.rearrange("p (r w) -> p r w", w=WW), srcv, AF.Copy)
                return go

            def p_c1d(g):
                def go():
                    t = pxca.tile([C, T2], bf16, tag="xc_a",
                                  name=f"xc_a_{s}")[:, :bw] if g == 0 else \
                        pxcb.tile([C, T2], bf16, tag="xc_b",
                                  name=f"xc_b_{s}")[:, :bw]
                    holder["xc_a" if g == 0 else "xc_b"] = t
                    ps = pbig.tile([C, T2], f32, tag="pbig",
                                   name=f"c1d_{s}_{g}")[:, :bw]
                    mmacc(ps, [(w1e_sb[:, g * 4 + k, :], xc2[:, k:k + bw])
                               for k in range(4)])
                    nc.scalar.activation(t[:], ps[:], AF.Silu,
                                         bias=b1d_sb[:, g:g + 1])
                return go

            def p_z():
                sz = psz.tile([C, T2], bf16, tag="sz", name=f"sz_{s}")[:, :bw]
                state[("sz", s)] = sz
                psgz = pbig.tile([C, T2], f32, tag="pbig", name=f"z_{s}")[:, :bw]
                mmacc(psgz, [(winz_sb[:], xc2[:, 3:3 + bw])])
                nc.scalar.activation(sz[:], psgz[:], AF.Silu)

            def p_xp():
                xc_a, xc_b = holder["xc_a"], holder["xc_b"]
                state[("xca", s)] = xc_a
                psx = pbig.tile([38, T2], f32, tag="pbig", name=f"xp_{s}")[:, :bw]
                mmacc(psx, [(wxp_sb[:, 0, :], xc_a[:]),
                            (wxp_sb[:, 1, :], xc_b[:])])
                x_dbl = pxd.tile([38, T2], bf16, tag="x_dbl",
                                 name=f"x_dbl_{s}")[:, :bw]
                nc.scalar.activation(x_dbl[:], psx[:], AF.Copy)
                nc.sync.dma_start(xdd[:, cs:ce], x_dbl[:])
                holder["x_dbl"] = x_dbl

            def p_dt():
                x_dbl = holder["x_dbl"]
                edt = ped.tile([C, T2], bf16, tag="edt", name=f"edt_{s}")[:, :bw]
                for hh, (c0, cw) in enumerate(chs):
                    psD = pbank.tile([C, T], f32, tag="pbank",
                                     name=f"psD_{s}_{hh}")
                    nc.tensor.matmul(psD[:, :cw], wdt_sb[:],
                                     x_dbl[:32, c0:c0 + cw])
                    nc.scalar.activation(edt[:, c0:c0 + cw], psD[:, :cw],
                                         AF.Exp, bias=bdt_sb[:])
                delta_c = dl.tile([C, T2], bf16, tag="delta",
                                  name=f"delta_{s}")[:, :bw]
                nc.scalar.activation(delta_c[:], edt[:], AF.Ln, bias=1.0)
                du_c = dl.tile([C, T2], bf16, tag="du", name=f"du_{s}")[:, :bw]
                nc.gpsimd.tensor_mul(du_c[:], delta_c[:], holder["xc_a"][:])
                state[("delta", s)] = delta_c
                state[("du", s)] = du_c

            def p_bc(i):
                def go():
                    bbcc = state.setdefault(("bbcc", s), [])
                    for n in (2 * i, 2 * i + 1):
                        t = bc.tile([C, 2, T2], bf16, tag="bc",
                                    name=f"bc_{s}_{n}")[:, :, :bw]
                        row = xdd[DTR + n: DTR + n + 1, cs:ce]
                        srcb = bass.AP(tensor=row.tensor, offset=row.offset,
                                       ap=[[0, C], [NST * L, 2], [1, bw]])
                        nc.sync.dma_start(t[:], srcb)
                        bbcc.append(t)
                return go

            return [p_dconv(0), p_dconv(1), p_dconv(2), p_c1d(0), p_c1d(1),
                    p_z, p_xp, p_dt] + [p_bc(i) for i in range(8)]

        def _scan(s, pieces):
            cs, bw = BLOCKS[s]
            chs = [(c0, min(T, bw - c0)) for c0 in range(0, bw, T)]
            if s >= 0:
                xc_a = state.pop(("xca", s))
                sz = state.pop(("sz", s))
                bbcc = state.pop(("bbcc", s))
                delta_c = state.pop(("delta", s))
                du_c = state.pop(("du", s))

                yPh = [psy.tile([C, T], f32, tag="psy", name=f"yP_{s}_{hh}")
                       for hh in range(len(chs))]
                # gpsimd runs in-order: issue all its dBu products up front so
                # it never stalls behind anything waiting on a scan.
                gp_dBu = {}
                for n in sorted(POOL_DBU):
                    dBu = gbp.tile([C, T2], bf16, tag="dBug",
                                   name=f"dBu_{s}_{n}")[:, :bw]
                    nc.gpsimd.tensor_mul(dBu[:], du_c[:], bbcc[n][:, 0, :])
                    gp_dBu[n] = dBu

            dAs = {}

            def emit_dA(n):
                dA = dAs[n] = dap.tile([C, T2], bf16, tag="dA",
                                       name=f"dA_{s}_{n}")[:, :bw]
                nc.scalar.activation(dA[:], delta_c[:], AF.Exp,
                                     scale=aneg_sb[:, n:n + 1])

            # dA emission schedule: front-load a few so the DVE never waits,
            # and leave a gap at n in 3..5 where the front's Silu group runs.
            dA_sched = {0: (0,), 1: (1,), 2: (2, 3, 4, 5, 6)}
            dA_sched.update({n: (n,) for n in range(7, NST)})

            for n in range(NST):
                if s >= 0:
                    for m in dA_sched.get(n, ()):
                        emit_dA(m)
                    if n in POOL_DBU:
                        dBu = gp_dBu[n]
                    else:
                        dBu = lp.tile([C, T2], bf16, tag="dBu",
                                      name=f"dBu_{s}_{n}")[:, :bw]
                        nc.vector.tensor_mul(dBu[:], du_c[:], bbcc[n][:, 0, :])

                    h = hp.tile([C, T2], bf16, tag="h", name=f"h_{s}_{n}")[:, :bw]
                    init = 0.0 if s == 0 else carry[:, n:n + 1]
                    nc.vector.tensor_tensor_scan(h[:], dAs[n][:], dBu[:], init,
                                                 OP.mult, OP.add)
                    if s < len(BLOCKS) - 1:
                        # software-DGE DMA: costs the Pool SEQ only ~25ns and
                        # casts bf16->f32; consumed a full block later, so the
                        # descriptor-gen + sem latency is immaterial.
                        nc.gpsimd.dma_start(carry[:, n:n + 1], h[:, bw - 1:bw])

                    tmp = lp.tile([C, T2], bf16, tag="tmp",
                                  name=f"tmp_{s}_{n}")[:, :bw]
                    nc.vector.tensor_mul(tmp[:], h[:], bbcc[n][:, 1, :])
                    for hh, (c0, cw) in enumerate(chs):
                        nc.tensor.matmul(yPh[hh][:, :cw], ident_sb[:],
                                         tmp[:, c0:c0 + cw],
                                         start=(n == 0), stop=False)
                if pieces is not None:
                    pieces[n]()

            if s < 0:
                return
            # ---- D*u into the same accumulators, gate, out_proj -----------
            for hh, (c0, cw) in enumerate(chs):
                sl = slice(c0, c0 + cw)
                nc.tensor.matmul(yPh[hh][:, :cw], ddiag_sb[:], xc_a[:, sl],
                                 start=False, stop=True)
                y2 = tl.tile([C, T], bf16, tag="y2", name=f"y2_{s}_{hh}")[:, :cw]
                if s == len(BLOCKS) - 1:
                    # drain: DVE is idle at the end; skip the Act->Pool hops
                    nc.vector.tensor_mul(y2[:], yPh[hh][:, :cw], sz[:, sl])
                else:
                    yPc = tl.tile([C, T], bf16, tag="yPc",
                                  name=f"yPc_{s}_{hh}")[:, :cw]
                    nc.scalar.activation(yPc[:], yPh[hh][:, :cw], AF.Copy)
                    nc.gpsimd.tensor_mul(y2[:], yPc[:], sz[:, sl])
                outP = pbank.tile([C, T], f32, tag="pbank", name=f"outP_{s}_{hh}")
                nc.tensor.matmul(outP[:, :cw], wout_sb[:], y2[:])
                osb = tl.tile([C, T], f32, tag="osb", name=f"osb_{s}_{hh}")[:, :cw]
                nc.scalar.activation(osb[:], outP[:, :cw], AF.Copy)
                nc.sync.dma_start(
                    dram["out_part"][:, cs + c0: cs + c0 + cw], osb[:])

        # Software pipeline, depth 2: x-block DMA at b (3x3 conv needs a
        # one-row halo), the front end at b-1 interleaved piecewise with the
        # scan stage at b-2.
        NBK = len(BLOCKS)
        for b in range(NBK + 2):
            if 0 < b < NBK:
                cs, bw = BLOCKS[b]
                nrow = bw // WW
                row0 = cs // WW
                dst = xp1[:, G + (row0 + 1) * PW + 1:
                          G + (row0 + nrow + 1) * PW + 1]
                dst = dst.rearrange("p (r w) -> p r w", w=PW)[:, :, 0:WW]
                nc.sync.dma_start(dst, dram["x"][:, cs:cs + bw]
                                  .rearrange("p (r w) -> p r w", w=WW))
            pieces = _front_pieces(b - 1) if 1 <= b <= NBK else None
            _scan(b - 2, pieces)


def _build_program():
    from concourse import bacc, tile, mybir

    # The act-table-load pass assigns each activation the FIRST table
    # containing its function; Exp and Ln then land in different tables and
    # every Exp<->Ln transition costs a 1.3us table reload. Restrict the
    # candidate set (during build only) to one table that has Exp+Ln+Copy
    # and the Silu table, so each block needs only 2 loads. Table ids keep
    # their act_info.json indices, so walrus lowering is unaffected.
    from concourse import hw_specs as _hw
    _orig_tabs = _hw.get_activation_tables
    _keep = {"natural_log_exp_and_others", "silu_and_others"}

    def _patched(arch):
        tabs = _orig_tabs(arch)
        return {name: (s if name in _keep else set())
                for name, s in tabs.items()}

    _hw.get_activation_tables = _patched
    bacc.get_activation_tables = _patched
    try:
        return _build_program_inner(bacc, tile, mybir)
    finally:
        _hw.get_activation_tables = _orig_tabs
        bacc.get_activation_tables = _orig_tabs


def _build_program_inner(bacc, tile, mybir):
    nc = bacc.Bacc("TRN2", target_bir_lowering=False, debug=False, num_devices=8)
    f32 = mybir.dt.float32
    bf16 = mybir.dt.bfloat16

    def din(name, shape, dtype=f32):
        return nc.dram_tensor(name, shape, dtype, kind="ExternalInput").ap()

    dram = {
        "x": din("x", (C, L), bf16),
        "wde": din("wde", (9, C, C), bf16),
        "w1e": din("w1e", (2, 4, C, C), bf16),
        "winz": din("winz", (C, C), bf16),
        "b1d": din("b1d", (C, 2)),
        "wxp": din("wxp", (2, C, 38), bf16),
        "wdt": din("wdt", (32, C), bf16),
        "bdt": din("bdt", (C, 1)),
        "wout": din("wout", (C, C), bf16),
        "aneg": din("aneg", (C, NST)),
        "ident": din("ident", (C, C), bf16),
        "ddiag": din("ddiag", (C, C), bf16),
        "out_part": nc.dram_tensor("out_part", (C, L), f32,
                                   kind="ExternalOutput").ap(),
    }

    with tile.TileContext(nc) as tc:
        _emit(tc, nc, mybir, dram)
    nc.compile()
    return nc


def get_program():
    if "nc" not in _CACHE:
        _CACHE["nc"] = _build_program()
    return _CACHE["nc"]


def make_core_inputs(inputs, b, half):
    import ml_dtypes

    bf = ml_dtypes.bfloat16
    perm = np.concatenate([
        np.arange(half * C, half * C + C),
        np.arange((1 - half) * C, (1 - half) * C + C),
    ])
    loc = perm[:C]

    a = np.exp(np.asarray(inputs["A_log"], np.float64))[loc].astype(np.float32)

    # fold 1x1 proj into the 3x3 conv: W_eff[o,c,tap] = sum_i dw[o,i,t] wp[i,c]
    dw = np.asarray(inputs["dconv_w"], np.float32)   # (96, 192, 3, 3)
    wp = np.asarray(inputs["proj_w"], np.float32)[:, :, 0, 0]  # (192, 96)
    wde = np.empty((9, C, C), np.float32)            # (tap, c_in, o)
    for tap in range(9):
        dy, dx = tap // 3, tap % 3
        wde[tap] = (dw[:, :, dy, dx] @ wp).T

    # fold conv1d into in_proj: W1eff[g,k][c,d] = win_g[d,c] * w1[g*96+d,k]
    w_in = np.asarray(inputs["in_proj_w"], np.float32)
    w1 = np.asarray(inputs["conv1d_w"], np.float32)[perm]   # (192, 4)
    w1e = np.empty((2, 4, C, C), np.float32)
    for g in range(2):
        wing = w_in[perm[g * C:(g + 1) * C]]         # (96 d, 96 c)
        for k in range(4):
            w1e[g, k] = (wing * w1[g * C:(g + 1) * C, k][:, None]).T
    winz = w_in[DI + loc].T                          # (96 c, 96 d)

    b1 = np.asarray(inputs["conv1d_b"], np.float32)[perm]
    b1d = np.stack([b1[:C], b1[C:]], axis=1)

    wxp_full = np.asarray(inputs["x_proj_w"], np.float32)[:, perm]  # (38, 192)
    wxp = np.stack([wxp_full[:, :C].T, wxp_full[:, C:].T], axis=0)

    wdt = np.zeros((32, C), np.float32)
    wdt[:DTR] = np.asarray(inputs["dt_proj_w"], np.float32)[loc].T

    return {
        "x": np.ascontiguousarray(
            np.asarray(inputs["x"], np.float32)[b].reshape(C, L)).astype(bf),
        "wde": np.ascontiguousarray(wde).astype(bf),
        "w1e": np.ascontiguousarray(w1e).astype(bf),
        "winz": np.ascontiguousarray(winz).astype(bf),
        "b1d": np.ascontiguousarray(b1d),
        "wxp": np.ascontiguousarray(wxp).astype(bf),
        "wdt": wdt.astype(bf),
        "bdt": np.asarray(inputs["dt_proj_b"], np.float32)[loc, None],
        "wout": np.ascontiguousarray(
            np.asarray(inputs["out_proj_w"], np.float32)[:, loc].T).astype(bf),
        "aneg": -a,
        "ident": np.eye(C, dtype=np.float32).astype(bf),
        "ddiag": np.diag(np.asarray(inputs["D"], np.float32)[loc]).astype(bf),
    }


def kernel(**inputs):
    from concourse import bass_utils

    nc = get_program()
    in_maps = [make_core_inputs(inputs, b, half)
               for b in range(4) for half in range(2)]
    res = bass_utils.run_bass_kernel_spmd(nc, in_maps, core_ids=list(range(8)))
    out = np.zeros((4, C, L), np.float32)
    for b in range(4):
        out[b] = res.results[2 * b]["out_part"] + res.results[2 * b + 1]["out_part"]
    return out.reshape(4, C, HH, WW)
